# revision 1
# baseline (speedup 1.0000x reference)
"""AriaTextMoELayer on 8 TRN2 NeuronCores — expert-parallel Bass kernel.

Strategy (hardcoded for E=8 experts, TOPK=2, H=1024, I=1024, ISH=2048,
B*S = 2048 tokens, 8 cores):
  - Core e owns expert e: fc1_w[e], fc2_w[e].
  - Shared-expert MLP is tensor-parallel on the intermediate dim:
    core e owns gate_w/up_w[:, 256e:256e+256] and down_w rows [256e:256e+256].
  - hidden_states (transposed to [H, N] on host) and w_router replicated.
  - On device, each core computes router logits for all tokens (fp32, exact),
    derives its expert's per-token top-2 softmax weight w_e with a closed form
    (w_e = [l_e >= m2] * sigmoid(2*l_e - m1 - m2)), runs its expert's SwiGLU
    MLP densely over all tokens (float32r matmuls), scales by w_e (so
    non-routed tokens contribute exactly 0), adds its shared-expert partial,
    and per-half-chunk ReduceScatters over token rows sum the 8 partials.
  - Host reassembles the shards.
"""
import sys

if "/opt/trn_rl_repo" not in sys.path:
    sys.path.insert(0, "/opt/trn_rl_repo")

import numpy as np

from concourse import bacc, bass, mybir, tile
from concourse.masks import make_identity

E = 8
H = 1024
I2 = 2048          # 2*I (fc1 output)
ISH_SH = 256       # shared intermediate shard per core
N = 2048           # tokens
NCORES = 8
TC = 512           # token chunk
NCHUNK = N // TC   # 4
KT = H // 128      # 8 contraction tiles
TT = TC // 128     # 4 token sub-tiles per chunk

F32 = mybir.dt.float32
F32R = mybir.dt.float32r
BF16 = mybir.dt.bfloat16
AX = mybir.AxisListType
OP = mybir.AluOpType
ACTF = mybir.ActivationFunctionType


def build():
    nc = bacc.Bacc(None, target_bir_lowering=False, debug=False)

    xT_d = nc.declare_dram_parameter("xT", [H, N], F32, isOutput=False)
    wr_d = nc.declare_dram_parameter("wr", [H, E], F32, isOutput=False)
    fc1_d = nc.declare_dram_parameter("fc1", [H, I2], F32, isOutput=False)
    fc2_d = nc.declare_dram_parameter("fc2", [H, H], F32, isOutput=False)
    gw_d = nc.declare_dram_parameter("gw", [H, ISH_SH], F32, isOutput=False)
    uw_d = nc.declare_dram_parameter("uw", [H, ISH_SH], F32, isOutput=False)
    dw_d = nc.declare_dram_parameter("dw", [ISH_SH, H], F32, isOutput=False)
    esel_d = nc.declare_dram_parameter("esel", [128, TT, E], F32, isOutput=False)
    # per (chunk, half): core r's ReduceScatter shard is [32 tokens, 2, 512]
    out_d = nc.declare_dram_parameter(
        "out", [NCHUNK, 2, 32, 2, 512], BF16, isOutput=True
    )

    with tile.TileContext(nc) as tc:
        with (
            tc.tile_pool(name="wpool", bufs=1) as wpool,
            tc.tile_pool(name="xpool", bufs=2) as xpool,
            tc.tile_pool(name="gpool", bufs=2) as gpool,
            tc.tile_pool(name="shpool", bufs=2) as shpool,
            tc.tile_pool(name="tmppool", bufs=2) as tmppool,
            tc.tile_pool(name="stpool", bufs=3) as stpool,
            tc.tile_pool(name="rpool", bufs=2) as rpool,
            tc.tile_pool(name="psab", bufs=3, space="PSUM") as psab,
            tc.tile_pool(name="psey", bufs=3, space="PSUM") as psey,
            tc.tile_pool(name="psr", bufs=1, space="PSUM") as psr,
            tc.tile_pool(name="dram", bufs=1, space="DRAM") as dram,
        ):
            # contiguous per-(chunk,half) collective buffers (bf16 on the wire;
            # separate tiles so Tile's DRAM dep tracking doesn't serialize
            # chunk c+1's writes behind chunk c's ReduceScatter reads)
            rs_in = [
                dram.tile(
                    [TT, 128, 2, 512], BF16, tag=f"rsin{c}", name=f"rsin{c}"
                )
                for c in range(NCHUNK)
            ]
            rs_out = [
                [
                    dram.tile(
                        [32, 2, 512],
                        BF16,
                        tag=f"rsout{c}_{h}",
                        name=f"rsout{c}_{h}",
                    )
                    for h in range(2)
                ]
                for c in range(NCHUNK)
            ]

            # ---- weights / inputs (DMA emission order = fetch priority) ----
            wr_t = wpool.tile([128, KT, E], F32)
            esel_t = wpool.tile([128, TT, E], F32)
            ident = wpool.tile([E, E], F32)
            nc.sync.dma_start(wr_t[:], wr_d[:].rearrange("(k p) e -> p k e", p=128))
            nc.sync.dma_start(esel_t[:], esel_d[:])
            make_identity(nc, ident[:])

            xT_src = xT_d[:].rearrange("(k p) t -> p k t", p=128)
            x0_t = xpool.tile([128, KT, TC], F32R, tag="x")
            nc.sync.dma_start(x0_t[:], xT_src[:, :, 0:TC].bitcast(F32R))

            fc1_t = wpool.tile([128, KT, I2], F32R)
            fc1_src = fc1_d[:].rearrange("(k p) o -> p k o", p=128)
            # column pair-groups: group g unlocks proj/gate tile pairs 2g,2g+1
            for g in range(4):
                nc.sync.dma_start(
                    fc1_t[:, :, g * 256 : (g + 1) * 256],
                    fc1_src[:, :, g * 256 : (g + 1) * 256].bitcast(F32R),
                )
                nc.sync.dma_start(
                    fc1_t[:, :, 1024 + g * 256 : 1024 + (g + 1) * 256],
                    fc1_src[:, :, 1024 + g * 256 : 1024 + (g + 1) * 256].bitcast(
                        F32R
                    ),
                )

            gw_t = wpool.tile([128, KT, ISH_SH], F32R)
            uw_t = wpool.tile([128, KT, ISH_SH], F32R)
            nc.sync.dma_start(
                gw_t[:], gw_d[:].rearrange("(k p) o -> p k o", p=128).bitcast(F32R)
            )
            nc.sync.dma_start(
                uw_t[:], uw_d[:].rearrange("(k p) o -> p k o", p=128).bitcast(F32R)
            )

            fc2_t = wpool.tile([128, KT, H], F32R)
            fc2_src = fc2_d[:].rearrange("(k p) o -> p k o", p=128)
            for k0 in range(0, KT, 4):
                nc.sync.dma_start(
                    fc2_t[:, k0 : k0 + 4, :],
                    fc2_src[:, k0 : k0 + 4, :].bitcast(F32R),
                )
            dw_t = wpool.tile([128, 2, H], F32R)
            nc.sync.dma_start(
                dw_t[:], dw_d[:].rearrange("(k p) o -> p k o", p=128).bitcast(F32R)
            )

            for c in range(NCHUNK):
                ts, te = c * TC, (c + 1) * TC

                if c == 0:
                    x_t = x0_t
                else:
                    x_t = xpool.tile([128, KT, TC], F32R, tag="x")
                    nc.sync.dma_start(x_t[:], xT_src[:, :, ts:te].bitcast(F32R))
                x_f32 = x_t[:].bitcast(F32)

                # ---- router: expert-major logits, then transpose ----
                lp = psr.tile([E, TC], F32, tag="r")
                for k in range(KT):
                    nc.tensor.matmul(
                        lp[:],
                        wr_t[:, k, :],
                        x_f32[:, k, :],
                        start=(k == 0),
                        stop=(k == KT - 1),
                    )
                l_em = tmppool.tile([E, TC], F32, tag="silu")
                nc.vector.tensor_copy(l_em[:], lp[:])
                logits = rpool.tile([128, TT, E], F32, tag="logits")
                for tt in range(TT):
                    ltp = psr.tile([128, E], F32, tag="rt")
                    nc.tensor.transpose(
                        ltp[:], l_em[:, tt * 128 : (tt + 1) * 128], ident[:]
                    )
                    nc.vector.tensor_copy(logits[:, tt, :], ltp[:])

                # ---- top-2 weight for this core's expert ----
                m8 = rpool.tile([128, TT, 8], F32, tag="m8")
                for tt in range(TT):
                    nc.vector.max(m8[:, tt, :], logits[:, tt, :])
                ltmp = rpool.tile([128, TT, E], F32, tag="ltmp")
                nc.vector.tensor_tensor(ltmp[:], logits[:], esel_t[:], OP.mult)
                le = rpool.tile([128, TT], F32, tag="le")
                nc.vector.tensor_reduce(le[:], ltmp[:], AX.X, OP.add)
                s12 = rpool.tile([128, TT], F32, tag="s12")
                nc.vector.tensor_tensor(
                    s12[:], m8[:, :, 0:1], m8[:, :, 1:2], OP.add
                )
                pre = rpool.tile([128, TT], F32, tag="pre")
                nc.vector.scalar_tensor_tensor(
                    pre[:], le[:], 2.0, s12[:], OP.mult, OP.subtract
                )
                sig = rpool.tile([128, TT], F32, tag="sig")
                nc.scalar.activation(sig[:], pre[:], ACTF.Sigmoid)
                ind = rpool.tile([128, TT], F32, tag="ind")
                nc.vector.tensor_tensor(ind[:], le[:], m8[:, :, 1:2], OP.is_ge)
                w_e = rpool.tile([128, TT], F32, tag="we")
                nc.vector.tensor_tensor(w_e[:], sig[:], ind[:], OP.mult)

                # ---- expert GEMM1 + SwiGLU -> G^T [128, KT(i), TC] f32r ----
                g_t = gpool.tile([128, KT, TC], F32R, tag="g")
                for j in range(KT):  # 8 proj/gate tile pairs
                    pa = psab.tile([128, TC], F32, tag="ab")
                    pb = psab.tile([128, TC], F32, tag="ab")
                    for k in range(KT):
                        nc.tensor.matmul(
                            pa[:],
                            fc1_t[:, k, j * 128 : (j + 1) * 128],
                            x_t[:, k, :],
                            start=(k == 0),
                            stop=(k == KT - 1),
                        )
                    for k in range(KT):
                        nc.tensor.matmul(
                            pb[:],
                            fc1_t[:, k, 1024 + j * 128 : 1024 + (j + 1) * 128],
                            x_t[:, k, :],
                            start=(k == 0),
                            stop=(k == KT - 1),
                        )
                    stmp = tmppool.tile([128, TC], F32, tag="silu")
                    nc.scalar.activation(stmp[:], pa[:], ACTF.Silu)
                    nc.vector.tensor_tensor(g_t[:, j, :], stmp[:], pb[:], OP.mult)

                # ---- shared gate/up -> sh^T [128, 2, TC] f32r ----
                sh_t = shpool.tile([128, 2, TC], F32R, tag="sh")
                for o2 in range(2):
                    pg = psab.tile([128, TC], F32, tag="ab")
                    pu = psab.tile([128, TC], F32, tag="ab")
                    for k in range(KT):
                        nc.tensor.matmul(
                            pg[:],
                            gw_t[:, k, o2 * 128 : (o2 + 1) * 128],
                            x_t[:, k, :],
                            start=(k == 0),
                            stop=(k == KT - 1),
                        )
                    for k in range(KT):
                        nc.tensor.matmul(
                            pu[:],
                            uw_t[:, k, o2 * 128 : (o2 + 1) * 128],
                            x_t[:, k, :],
                            start=(k == 0),
                            stop=(k == KT - 1),
                        )
                    stmp = tmppool.tile([128, TC], F32, tag="silu")
                    nc.scalar.activation(stmp[:], pg[:], ACTF.Silu)
                    nc.vector.tensor_tensor(sh_t[:, o2, :], stmp[:], pu[:], OP.mult)

                # ---- GEMM2(+down) token-major, scale expert part by w_e ----
                for tt in range(TT):
                    for hh in range(2):
                        hs, he = hh * 512, (hh + 1) * 512
                        pe = psey.tile([128, 512], F32, tag="ey")
                        for i in range(KT):
                            nc.tensor.matmul(
                                pe[:],
                                g_t[:, i, tt * 128 : (tt + 1) * 128],
                                fc2_t[:, i, hs:he],
                                start=(i == 0),
                                stop=(i == KT - 1),
                            )
                        ps = psey.tile([128, 512], F32, tag="ey")
                        for i2 in range(2):
                            nc.tensor.matmul(
                                ps[:],
                                sh_t[:, i2, tt * 128 : (tt + 1) * 128],
                                dw_t[:, i2, hs:he],
                                start=(i2 == 0),
                                stop=(i2 == 1),
                            )
                        stage_f = stpool.tile([128, 512], F32, tag="stf")
                        nc.vector.tensor_scalar(
                            stage_f[:], pe[:], w_e[:, tt : tt + 1], None, OP.mult
                        )
                        stage_b = stpool.tile([128, 512], BF16, tag="stb")
                        nc.vector.tensor_tensor(
                            stage_b[:], stage_f[:], ps[:], OP.add
                        )
                        nc.sync.dma_start(rs_in[c][tt, :, hh, :], stage_b[:])

                    # after each half's stages are out, ReduceScatter that half
                    if tt == 1 or tt == 3:
                        ha = tt // 2
                        nc.gpsimd.collective_compute(
                            "ReduceScatter",
                            OP.add,
                            replica_groups=[list(range(NCORES))],
                            ins=[rs_in[c][2 * ha : 2 * ha + 2].opt()],
                            outs=[rs_out[c][ha].opt()],
                        )
                        nc.sync.dma_start(out_d[c, ha], rs_out[c][ha][:])

    nc.compile()
    return nc


_CACHED = {}


def _prep_in_maps(hidden_states, w_router, fc1_w, fc2_w, gate_w, up_w, down_w):
    xT = np.ascontiguousarray(
        hidden_states.reshape(-1, H).T.astype(np.float32)
    )  # [H, N]
    in_maps = []
    for e in range(NCORES):
        esel = np.zeros((128, TT, E), np.float32)
        esel[:, :, e] = 1.0
        in_maps.append(
            {
                "xT": xT,
                "wr": np.ascontiguousarray(w_router, np.float32),
                "fc1": np.ascontiguousarray(fc1_w[e], np.float32),
                "fc2": np.ascontiguousarray(fc2_w[e], np.float32),
                "gw": np.ascontiguousarray(gate_w[:, e * 256 : (e + 1) * 256]),
                "uw": np.ascontiguousarray(up_w[:, e * 256 : (e + 1) * 256]),
                "dw": np.ascontiguousarray(down_w[e * 256 : (e + 1) * 256, :]),
                "esel": esel,
            }
        )
    return in_maps


def _assemble(results, orig_shape):
    # Core r's shard of (chunk c, half ha) = [32 tokens, 2 h-halves, 512]:
    # tokens [c*512 + (2*ha + r//4)*128 + 32*(r%4) + i], h cols [hh*512 + j].
    full = np.empty((N, H), np.float32)
    for r, res in enumerate(results):
        o = np.asarray(res["out"]).astype(np.float32).reshape(NCHUNK, 2, 32, 2, 512)
        for c in range(NCHUNK):
            for ha in range(2):
                t0 = c * TC + (2 * ha + r // 4) * 128 + 32 * (r % 4)
                blk = o[c, ha]  # [32, 2, 512]
                full[t0 : t0 + 32, 0:512] = blk[:, 0, :]
                full[t0 : t0 + 32, 512:1024] = blk[:, 1, :]
    return full.reshape(orig_shape)


def kernel(hidden_states, w_router, fc1_w, fc2_w, gate_w, up_w, down_w):
    from concourse.bass_utils import run_bass_kernel_spmd

    if "nc" not in _CACHED:
        _CACHED["nc"] = build()
    nc = _CACHED["nc"]
    in_maps = _prep_in_maps(
        hidden_states, w_router, fc1_w, fc2_w, gate_w, up_w, down_w
    )
    res = run_bass_kernel_spmd(nc, in_maps, core_ids=list(range(NCORES)))
    return _assemble(res.results, hidden_states.shape)

